# revision 7
# baseline (speedup 1.0000x reference)
"""Bidirectional complex-diagonal LRU on 8 TRN2 cores — v2.

Key structure (vs the v1 kernel):
  - No data reversal anywhere: backward chunks scan via reversed APs
    (time-descending recurrence); tables for bwd chunks are host-baked with
    phase theta*(L-1-t) in forward storage order. All DVE tensor ops hit the
    2x perf mode (stride +1, fp16, SBUF).
  - Sign juggling (host-negated B_im, +C1, negated-w-im scan) turns one
    combine per complex multiply into a pure ADD -> rides SWDGE accum DMA.
    Per (chunk, slab): 8 muls + 2 subs on DVE, 2 accum-DMAs.
  - Scans run in place (w overwrites its v slab); cross-slab chain init is
    an AP into the neighbouring column.
  - Middle slabs (away from the chain heads) process twiddle/untwiddle at
    2048 width: ~12% cheaper per element and half the instructions/accums.
  - y computed transposed (yT = C^T x, [H, L], fp16); host transposes back.
    x chunks are the matmul MOVING operand -> stationary C chunks reused.
  - y(3) opens early with its bwd half (ready after step 0); fwd closes it.

Self-contained: hardcodes B=8, L=4096, H=N=512, 8 cores.
"""

import numpy as np
from contextlib import ExitStack

import concourse.bass as bass
import concourse.bacc as bacc
import concourse.mybir as mybir
import concourse.tile as tile

P = 128
L = 4096
H = 512
N = 512
BSZ = 8
SEG = 512
SLAB = 1024
NSLAB = L // SLAB        # 4
KH = H // P              # 4
NCH = 2 * N // P         # 8 real-channel planes (re 0..3, imN 4..7)
CCH = N // P             # 4 complex chunks (0,1 fwd; 2,3 bwd)

F16 = mybir.dt.float16
F32 = mybir.dt.float32
MULT = mybir.AluOpType.mult
ADD = mybir.AluOpType.add

_CACHED = None


def build_nc():
    nc = bacc.Bacc("TRN2", target_bir_lowering=False, debug=False)
    uT = nc.declare_dram_parameter("uT", [H, L], F16, isOutput=False)
    cosT = nc.declare_dram_parameter("cosT", [N, L], F16, isOutput=False)
    sinT = nc.declare_dram_parameter("sinT", [N, L], F16, isOutput=False)
    rdec = nc.declare_dram_parameter("rdec", [P, CCH], F32, isOutput=False)
    Bcat = nc.declare_dram_parameter("Bcat", [H, 2 * N], F16, isOutput=False)
    Ccat = nc.declare_dram_parameter("Ccat", [2 * N, H], F16, isOutput=False)
    yT = nc.declare_dram_parameter("yT", [H, L], F16, isOutput=True)

    with tile.TileContext(nc) as tc, ExitStack() as ctx:
        const = ctx.enter_context(tc.tile_pool(name="const", bufs=1))
        big = ctx.enter_context(tc.tile_pool(name="big", bufs=1))
        upool = ctx.enter_context(tc.tile_pool(name="upool", bufs=1))
        scr1 = ctx.enter_context(tc.tile_pool(name="scr1", bufs=12))
        ysb = ctx.enter_context(tc.tile_pool(name="ysb", bufs=2))
        bup = ctx.enter_context(tc.tile_pool(name="bup", bufs=2, space="PSUM"))

        u_t = [upool.tile([P, 2 * SLAB], F16, tag=f"uT{k}", name=f"uT{k}")
               for k in range(KH)]
        cosb = [big.tile([P, L], F16, tag=f"cos{c}", name=f"cos{c}")
                for c in range(CCH)]
        sinb = [big.tile([P, L], F16, tag=f"sin{c}", name=f"sin{c}")
                for c in range(CCH)]
        v = [big.tile([P, L], F16, tag=f"v{j}", name=f"v{j}") for j in range(NCH)]
        bmat = [const.tile([P, 2 * N], F16, tag=f"B{k}", name=f"Bm{k}")
                for k in range(KH)]
        cmat = [const.tile([P, H], F16, tag=f"C{k}", name=f"Cm{k}")
                for k in range(NCH)]
        rdec_t = const.tile([P, CCH], F32, tag="rdec", name="rdec_t")

        # ---- constant DMAs: Bcat + u wave 1 first so Bu starts ASAP
        nc.sync.dma_start(rdec_t[:], rdec[:])
        for k in range(KH):
            nc.sync.dma_start(bmat[k][:], Bcat[k * P:(k + 1) * P, :])
        # u piece A: t cols [0:1024) + [3072:4096) — on the Activation
        # queue so they race ahead of the (big) table loads on sync
        for k in range(KH):
            nc.scalar.dma_start(u_t[k][:, 0:SLAB],
                                uT[k * P:(k + 1) * P, 0:SLAB])
        for k in range(KH):
            nc.scalar.dma_start(u_t[k][:, SLAB:2 * SLAB],
                              uT[k * P:(k + 1) * P, 3 * SLAB:4 * SLAB])
        # tables: slab 0 (fwd) + slab 3 (bwd scan heads) first, on the Pool
        # queue (idle until the first accum anyway)
        for c in range(CCH):
            sl0 = slice(0, SLAB) if c < 2 else slice(3 * SLAB, 4 * SLAB)
            nc.gpsimd.dma_start(cosb[c][:, sl0], cosT[c * P:(c + 1) * P, sl0])
            nc.gpsimd.dma_start(sinb[c][:, sl0], sinT[c * P:(c + 1) * P, sl0])
        for c in range(CCH):
            slr = slice(SLAB, 4 * SLAB) if c < 2 else slice(0, 3 * SLAB)
            nc.sync.dma_start(cosb[c][:, slr], cosT[c * P:(c + 1) * P, slr])
            nc.sync.dma_start(sinb[c][:, slr], sinT[c * P:(c + 1) * P, slr])
        for k in range(NCH):
            nc.sync.dma_start(cmat[k][:], Ccat[k * P:(k + 1) * P, :])

        # ---- Phase A: Bu matmuls ----
        # group = 4 tsegs of one plane in a 4-bank PSUM tile; k-inner reuses
        # the stationary across the 4 tsegs.
        # utile cols: piece A: slab0 -> [0:1024), slab3 -> [1024:2048)
        #             piece B: slab1 -> [0:1024), slab2 -> [1024:2048)
        NCH_ORDER = [0, 4, 1, 5, 2, 6, 3, 7]   # pair c ready in order 0,1,2,3

        def bu_group(nch, piece):
            # piece 0 covers slabs (0, 3); piece 1 covers (1, 2)
            slabs = (0, 3) if piece == 0 else (1, 2)
            ps = bup.tile([P, 4 * SEG], F32, tag="bup", name=f"bu{nch}_{piece}")
            for k in range(KH):
                for i in range(4):
                    nc.tensor.matmul(
                        ps[:, i * SEG:(i + 1) * SEG],
                        bmat[k][:, nch * P:(nch + 1) * P],
                        u_t[k][:, i * SEG:(i + 1) * SEG],
                        start=(k == 0), stop=(k == KH - 1),
                    )
            nc.scalar.copy(v[nch][:, slabs[0] * SLAB:(slabs[0] + 1) * SLAB],
                           ps[:, 0:SLAB])
            nc.scalar.copy(v[nch][:, slabs[1] * SLAB:(slabs[1] + 1) * SLAB],
                           ps[:, SLAB:2 * SLAB])

        for nch in NCH_ORDER:
            bu_group(nch, 0)
        # u piece B: slab1 cols -> utile[0:1024), slab2 cols -> [1024:2048)
        for k in range(KH):
            nc.scalar.dma_start(u_t[k][:, 0:SLAB],
                              uT[k * P:(k + 1) * P, SLAB:2 * SLAB])
        for k in range(KH):
            nc.scalar.dma_start(u_t[k][:, SLAB:2 * SLAB],
                              uT[k * P:(k + 1) * P, 2 * SLAB:3 * SLAB])
        for nch in NCH_ORDER:
            bu_group(nch, 1)

        # ---- Phases B/C/D ----
        def r_ap(c, w):
            return rdec_t[:, c:c + 1].broadcast_to((P, w))

        def twiddle_in(c, sb, nsl=1):
            """Twiddle slabs [sb, sb+nsl) of chunk c (width nsl*1024)."""
            w = nsl * SLAB
            jre, jim = c, c + CCH
            sl = slice(sb * SLAB, sb * SLAB + w)
            cs, sn = cosb[c][:, sl], sinb[c][:, sl]
            vre, vim = v[jre][:, sl], v[jim][:, sl]   # vim holds -Bu_im
            pool = scr1
            sA = pool.tile([P, w], F16, tag="s", name="sA")[:]
            sB = pool.tile([P, w], F16, tag="s", name="sB")[:]
            sC = pool.tile([P, w], F16, tag="s", name="sC")[:]
            nc.vector.tensor_mul(sA, cs, vre)        # cs*vre
            nc.vector.tensor_mul(sB, sn, vim)        # sn*vimN
            nc.vector.tensor_mul(sC, cs, vim)        # cs*vimN
            nc.vector.tensor_mul(vim, sn, vre)       # sn*vre -> vim slab
            nc.vector.tensor_sub(vre, sA, sB)        # vre' = a1-a2
            # vim'N = a3 + a4   (accum: vim (holds a4) += sC)
            nc.gpsimd.dma_start(vim, sC, accum_op=ADD)

        def scan_pair(c, sb):
            jre, jim = c, c + CCH
            sl = slice(sb * SLAB, (sb + 1) * SLAB)
            vre, vim = v[jre][:, sl], v[jim][:, sl]
            if c < 2:
                init_r = (0.0 if sb == 0
                          else v[jre][:, sb * SLAB - 1:sb * SLAB])
                init_i = (0.0 if sb == 0
                          else v[jim][:, sb * SLAB - 1:sb * SLAB])
                nc.vector.tensor_tensor_scan(vre, r_ap(c, SLAB), vre, init_r,
                                             op0=MULT, op1=ADD)
                nc.vector.tensor_tensor_scan(vim, r_ap(c, SLAB), vim, init_i,
                                             op0=MULT, op1=ADD)
            else:
                e = (sb + 1) * SLAB
                init_r = 0.0 if sb == NSLAB - 1 else v[jre][:, e:e + 1]
                init_i = 0.0 if sb == NSLAB - 1 else v[jim][:, e:e + 1]
                nc.vector.tensor_tensor_scan(vre[:, ::-1], r_ap(c, SLAB),
                                             vre[:, ::-1], init_r,
                                             op0=MULT, op1=ADD)
                nc.vector.tensor_tensor_scan(vim[:, ::-1], r_ap(c, SLAB),
                                             vim[:, ::-1], init_i,
                                             op0=MULT, op1=ADD)

        def untwiddle(c, sb, nsl=1):
            w = nsl * SLAB
            jre, jim = c, c + CCH
            sl = slice(sb * SLAB, sb * SLAB + w)
            cs, sn = cosb[c][:, sl], sinb[c][:, sl]
            wr, wi = v[jre][:, sl], v[jim][:, sl]   # in-place scan outputs
            pool = scr1
            s1 = pool.tile([P, w], F16, tag="s", name="s1")[:]
            s2 = pool.tile([P, w], F16, tag="s", name="s2")[:]
            s3 = pool.tile([P, w], F16, tag="s", name="s3")[:]
            nc.vector.tensor_mul(s1, cs, wr)         # cs*wr
            nc.vector.tensor_mul(s2, cs, wi)         # cs*wiN
            nc.vector.tensor_mul(cs, sn, wi)         # sn*wiN -> cs slab
            nc.vector.tensor_mul(s3, sn, wr)         # sn*wr
            # ximN = b4 - b3 -> sn slab  (v slabs keep w for chain inits)
            nc.vector.tensor_sub(sn, s2, s3)
            # xre = b2 + b1 -> cs slab  (accum: cs (holds b2) += s1)
            nc.gpsimd.dma_start(cs, s1, accum_op=ADD)

        # x source for yT matmul: plane k, slab sb, window tw (512 cols)
        def x_src(k, sb, tw):
            col = sb * SLAB + tw * SEG
            src = cosb[k] if k < CCH else sinb[k - CCH]
            return src[:, col:col + SEG]

        FWD_K = (0, 1, 4, 5)   # x chunks from fwd untwiddles
        BWD_K = (2, 3, 6, 7)
        _ytiles = {}

        def yT_slab_half(sb, ks, first):
            # two H-chunks share one 4-bank PSUM tile (halves of the free dim)
            for jp in range(H // P // 2):
                if first:
                    ps = bup.tile([P, 2 * SLAB], F32, tag="bup",
                                  name=f"y{sb}_{jp}")
                    _ytiles[(sb, jp)] = ps
                else:
                    ps = _ytiles[(sb, jp)]
                for jo in range(2):
                    j = 2 * jp + jo
                    for ki, k in enumerate(ks):
                        for tw in range(SLAB // SEG):
                            nc.tensor.matmul(
                                ps[:, jo * SLAB + tw * SEG:
                                   jo * SLAB + (tw + 1) * SEG],
                                cmat[k][:, j * P:(j + 1) * P],
                                x_src(k, sb, tw),
                                start=(first and ki == 0),
                                stop=((not first) and ki == len(ks) - 1),
                            )
                if not first:
                    yt = ysb.tile([P, 2 * SLAB], F16, tag="y", name="yt")
                    nc.scalar.copy(yt[:], ps[:])
                    for jo in range(2):
                        j = 2 * jp + jo
                        nc.sync.dma_start(
                            yT[j * P:(j + 1) * P, sb * SLAB:(sb + 1) * SLAB],
                            yt[:, jo * SLAB:(jo + 1) * SLAB])

        def yT_slab(sb):
            yT_slab_half(sb, FWD_K, True)
            yT_slab_half(sb, BWD_K, False)

        # ---- schedule: step s = fwd slab s (c=0,1), bwd slab 3-s;
        # wave emission, software-pipelined one step deep.
        def slab_of(c, s):
            return s if c < 2 else NSLAB - 1 - s

        def tw_wave(s):
            for c in range(CCH):
                twiddle_in(c, slab_of(c, s))

        def scan_wave(s):
            for c in range(CCH):
                scan_pair(c, slab_of(c, s))

        def untw_wave(s):
            for c in range(CCH):
                untwiddle(c, slab_of(c, s))

        tw_wave(0)
        scan_wave(0)
        tw_wave(1)
        untw_wave(0)
        scan_wave(1)
        tw_wave(2)
        untw_wave(1)
        scan_wave(2)
        tw_wave(3)
        untw_wave(2)
        yT_slab(1)
        yT_slab(2)

        # ---- tail: finish the bwd chain first so y(0) overlaps the fwd
        # finish; fwd slab 3 is half-pipelined so only one 512-window of
        # y(3) trails the last DVE op.
        def scan_half_fwd(c, h):
            jre, jim = c, c + CCH
            c0 = 3 * SLAB + h * SEG
            sl = slice(c0, c0 + SEG)
            init_r = v[jre][:, c0 - 1:c0]
            init_i = v[jim][:, c0 - 1:c0]
            nc.vector.tensor_tensor_scan(v[jre][:, sl], r_ap(c, SEG),
                                         v[jre][:, sl], init_r,
                                         op0=MULT, op1=ADD)
            nc.vector.tensor_tensor_scan(v[jim][:, sl], r_ap(c, SEG),
                                         v[jim][:, sl], init_i,
                                         op0=MULT, op1=ADD)

        def untw_half_fwd(c, h):
            jre, jim = c, c + CCH
            c0 = 3 * SLAB + h * SEG
            sl = slice(c0, c0 + SEG)
            cs, sn = cosb[c][:, sl], sinb[c][:, sl]
            wr, wi = v[jre][:, sl], v[jim][:, sl]
            s1 = scr1.tile([P, SEG], F16, tag="sh", name="s1h")[:]
            s2 = scr1.tile([P, SEG], F16, tag="sh", name="s2h")[:]
            s3 = scr1.tile([P, SEG], F16, tag="sh", name="s3h")[:]
            nc.vector.tensor_mul(s1, cs, wr)
            nc.vector.tensor_mul(s2, cs, wi)
            nc.vector.tensor_mul(cs, sn, wi)
            nc.vector.tensor_mul(s3, sn, wr)
            nc.vector.tensor_sub(sn, s2, s3)
            nc.gpsimd.dma_start(cs, s1, accum_op=ADD)

        def y3_fwd_tw(tw, last):
            for jp in range(H // P // 2):
                ps = _ytiles[(3, jp)]
                for jo in range(2):
                    j = 2 * jp + jo
                    for ki, k in enumerate(FWD_K):
                        nc.tensor.matmul(
                            ps[:, jo * SLAB + tw * SEG:
                               jo * SLAB + (tw + 1) * SEG],
                            cmat[k][:, j * P:(j + 1) * P],
                            x_src(k, 3, tw),
                            start=False, stop=(ki == len(FWD_K) - 1),
                        )
                if last:
                    yt = ysb.tile([P, 2 * SLAB], F16, tag="y", name="yt")
                    nc.scalar.copy(yt[:], ps[:])
                    for jo in range(2):
                        j = 2 * jp + jo
                        nc.sync.dma_start(
                            yT[j * P:(j + 1) * P, 3 * SLAB:4 * SLAB],
                            yt[:, jo * SLAB:(jo + 1) * SLAB])

        scan_pair(2, 0)
        scan_pair(3, 0)
        untwiddle(2, 0)
        untwiddle(3, 0)
        yT_slab(0)
        scan_half_fwd(0, 0)
        scan_half_fwd(1, 0)
        yT_slab_half(3, BWD_K, True)
        scan_half_fwd(0, 1)
        scan_half_fwd(1, 1)
        untw_half_fwd(0, 0)
        untw_half_fwd(1, 0)
        y3_fwd_tw(0, False)
        untw_half_fwd(0, 1)
        untw_half_fwd(1, 1)
        y3_fwd_tw(1, True)

    nc.compile()
    return nc


def prepare_inputs(u, lengths, nu_log, theta_log, B, C):
    """Host-side prep. Heavy math in fp64 for accuracy."""
    u = np.asarray(u)
    lengths = np.asarray(lengths)
    nu = np.exp(np.asarray(nu_log, np.float64))
    theta = np.exp(np.asarray(theta_log, np.float64))
    r = np.exp(-nu)                                    # |lam|, (N,)

    j = np.arange(L, dtype=np.float64)
    half = N // 2
    # fwd channels: phase theta*t; bwd channels: phase theta*(L-1-t)
    phase = np.empty((N, L), np.float64)
    phase[:half] = theta[:half, None] * j[None, :]
    phase[half:] = theta[half:, None] * (L - 1 - j)[None, :]
    phase = np.mod(phase, 2 * np.pi)
    cos_base = np.cos(phase).astype(np.float16)
    sin_base = np.sin(phase).astype(np.float16)

    Bcat = np.empty((H, 2 * N), np.float16)
    Bcat[:, :N] = np.asarray(B)[..., 0]
    Bcat[:, N:] = -np.asarray(B)[..., 1]               # negated B_im
    Ccat = np.empty((2 * N, H), np.float16)
    Ccat[:N] = np.asarray(C)[0]
    Ccat[N:] = np.asarray(C)[1]                        # +C1 (x_imN carries -)
    rdec = r.reshape(CCH, P).T.astype(np.float32).copy()   # (128, 4)

    in_maps = []
    for b in range(BSZ):
        ln = int(lengths[b])
        ub = np.array(u[b], np.float32)
        if ln < L:
            ub[ln:, :] = 0.0
        uTh = np.ascontiguousarray(ub.T.astype(np.float16))
        cosb = cos_base.copy()
        sinb = sin_base.copy()
        if ln < L:
            cosb[:, ln:] = 0
            sinb[:, ln:] = 0
        in_maps.append({
            "uT": uTh, "cosT": cosb, "sinT": sinb,
            "rdec": rdec, "Bcat": Bcat, "Ccat": Ccat,
        })
    return in_maps


def kernel(u, lengths, nu_log, theta_log, B, C):
    global _CACHED
    from concourse.bass_utils import run_bass_kernel_spmd
    in_maps = prepare_inputs(u, lengths, nu_log, theta_log, B, C)
    if _CACHED is None:
        _CACHED = build_nc()
    res = run_bass_kernel_spmd(_CACHED, in_maps, list(range(BSZ)))
    y = np.stack(
        [res.results[i]["yT"].T.astype(np.float32) for i in range(BSZ)],
        axis=0)
    return y


# revision 8
# speedup vs baseline: 1.0157x; 1.0157x over previous
"""Bidirectional complex-diagonal LRU on 8 TRN2 cores — v2.

Key structure (vs the v1 kernel):
  - No data reversal anywhere: backward chunks scan via reversed APs
    (time-descending recurrence); tables for bwd chunks are host-baked with
    phase theta*(L-1-t) in forward storage order. All DVE tensor ops hit the
    2x perf mode (stride +1, fp16, SBUF).
  - Sign juggling (host-negated B_im, +C1, negated-w-im scan) turns one
    combine per complex multiply into a pure ADD -> rides SWDGE accum DMA.
    Per (chunk, slab): 8 muls + 2 subs on DVE, 2 accum-DMAs.
  - Scans run in place (w overwrites its v slab); cross-slab chain init is
    an AP into the neighbouring column.
  - Middle slabs (away from the chain heads) process twiddle/untwiddle at
    2048 width: ~12% cheaper per element and half the instructions/accums.
  - y computed transposed (yT = C^T x, [H, L], fp16); host transposes back.
    x chunks are the matmul MOVING operand -> stationary C chunks reused.
  - y(3) opens early with its bwd half (ready after step 0); fwd closes it.

Self-contained: hardcodes B=8, L=4096, H=N=512, 8 cores.
"""

import numpy as np
from contextlib import ExitStack

import concourse.bass as bass
import concourse.bacc as bacc
import concourse.mybir as mybir
import concourse.tile as tile

P = 128
L = 4096
H = 512
N = 512
BSZ = 8
SEG = 512
SLAB = 1024
NSLAB = L // SLAB        # 4
KH = H // P              # 4
NCH = 2 * N // P         # 8 real-channel planes (re 0..3, imN 4..7)
CCH = N // P             # 4 complex chunks (0,1 fwd; 2,3 bwd)

F16 = mybir.dt.float16
F32 = mybir.dt.float32
MULT = mybir.AluOpType.mult
ADD = mybir.AluOpType.add

_CACHED = None


def build_nc():
    nc = bacc.Bacc("TRN2", target_bir_lowering=False, debug=False)
    uT = nc.declare_dram_parameter("uT", [H, L], F16, isOutput=False)
    cosT = nc.declare_dram_parameter("cosT", [N, L], F16, isOutput=False)
    sinT = nc.declare_dram_parameter("sinT", [N, L], F16, isOutput=False)
    rdec = nc.declare_dram_parameter("rdec", [P, CCH], F32, isOutput=False)
    Bcat = nc.declare_dram_parameter("Bcat", [H, 2 * N], F16, isOutput=False)
    Ccat = nc.declare_dram_parameter("Ccat", [2 * N, H], F16, isOutput=False)
    yT = nc.declare_dram_parameter("yT", [H, L], F16, isOutput=True)

    with tile.TileContext(nc) as tc, ExitStack() as ctx:
        const = ctx.enter_context(tc.tile_pool(name="const", bufs=1))
        big = ctx.enter_context(tc.tile_pool(name="big", bufs=1))
        upool = ctx.enter_context(tc.tile_pool(name="upool", bufs=1))
        scr1 = ctx.enter_context(tc.tile_pool(name="scr1", bufs=12))
        ysb = ctx.enter_context(tc.tile_pool(name="ysb", bufs=2))
        bup = ctx.enter_context(tc.tile_pool(name="bup", bufs=2, space="PSUM"))

        u_t = [upool.tile([P, 2 * SLAB], F16, tag=f"uT{k}", name=f"uT{k}")
               for k in range(KH)]
        cosb = [big.tile([P, L], F16, tag=f"cos{c}", name=f"cos{c}")
                for c in range(CCH)]
        sinb = [big.tile([P, L], F16, tag=f"sin{c}", name=f"sin{c}")
                for c in range(CCH)]
        v = [big.tile([P, L], F16, tag=f"v{j}", name=f"v{j}") for j in range(NCH)]
        bmat = [const.tile([P, 2 * N], F16, tag=f"B{k}", name=f"Bm{k}")
                for k in range(KH)]
        cmat = [const.tile([P, H], F16, tag=f"C{k}", name=f"Cm{k}")
                for k in range(NCH)]
        rdec_t = const.tile([P, CCH], F32, tag="rdec", name="rdec_t")

        # ---- constant DMAs: Bcat + u wave 1 first so Bu starts ASAP
        nc.sync.dma_start(rdec_t[:], rdec[:])
        for k in range(KH):
            nc.sync.dma_start(bmat[k][:], Bcat[k * P:(k + 1) * P, :])
        # u piece A: t cols [0:1024) + [3072:4096)
        for k in range(KH):
            nc.sync.dma_start(u_t[k][:, 0:SLAB], uT[k * P:(k + 1) * P, 0:SLAB])
        for k in range(KH):
            nc.sync.dma_start(u_t[k][:, SLAB:2 * SLAB],
                              uT[k * P:(k + 1) * P, 3 * SLAB:4 * SLAB])
        # tables: slab 0 (fwd) + slab 3 (bwd scan heads) first
        for c in range(CCH):
            sl0 = slice(0, SLAB) if c < 2 else slice(3 * SLAB, 4 * SLAB)
            nc.sync.dma_start(cosb[c][:, sl0], cosT[c * P:(c + 1) * P, sl0])
            nc.sync.dma_start(sinb[c][:, sl0], sinT[c * P:(c + 1) * P, sl0])
        for c in range(CCH):
            slr = slice(SLAB, 4 * SLAB) if c < 2 else slice(0, 3 * SLAB)
            nc.sync.dma_start(cosb[c][:, slr], cosT[c * P:(c + 1) * P, slr])
            nc.sync.dma_start(sinb[c][:, slr], sinT[c * P:(c + 1) * P, slr])
        for k in range(NCH):
            nc.sync.dma_start(cmat[k][:], Ccat[k * P:(k + 1) * P, :])

        # ---- Phase A: Bu matmuls ----
        # group = 4 tsegs of one plane in a 4-bank PSUM tile; k-inner reuses
        # the stationary across the 4 tsegs.
        # utile cols: piece A: slab0 -> [0:1024), slab3 -> [1024:2048)
        #             piece B: slab1 -> [0:1024), slab2 -> [1024:2048)
        NCH_ORDER = [0, 4, 1, 5, 2, 6, 3, 7]   # pair c ready in order 0,1,2,3

        def bu_group(nch, piece):
            # piece 0 covers slabs (0, 3); piece 1 covers (1, 2)
            slabs = (0, 3) if piece == 0 else (1, 2)
            ps = bup.tile([P, 4 * SEG], F32, tag="bup", name=f"bu{nch}_{piece}")
            for k in range(KH):
                for i in range(4):
                    nc.tensor.matmul(
                        ps[:, i * SEG:(i + 1) * SEG],
                        bmat[k][:, nch * P:(nch + 1) * P],
                        u_t[k][:, i * SEG:(i + 1) * SEG],
                        start=(k == 0), stop=(k == KH - 1),
                    )
            nc.scalar.copy(v[nch][:, slabs[0] * SLAB:(slabs[0] + 1) * SLAB],
                           ps[:, 0:SLAB])
            nc.scalar.copy(v[nch][:, slabs[1] * SLAB:(slabs[1] + 1) * SLAB],
                           ps[:, SLAB:2 * SLAB])

        for nch in NCH_ORDER:
            bu_group(nch, 0)
        # u piece B: slab1 cols -> utile[0:1024), slab2 cols -> [1024:2048)
        for k in range(KH):
            nc.sync.dma_start(u_t[k][:, 0:SLAB],
                              uT[k * P:(k + 1) * P, SLAB:2 * SLAB])
        for k in range(KH):
            nc.sync.dma_start(u_t[k][:, SLAB:2 * SLAB],
                              uT[k * P:(k + 1) * P, 2 * SLAB:3 * SLAB])
        for nch in NCH_ORDER:
            bu_group(nch, 1)

        # ---- Phases B/C/D ----
        def r_ap(c, w):
            return rdec_t[:, c:c + 1].broadcast_to((P, w))

        def twiddle_in(c, sb, nsl=1):
            """Twiddle slabs [sb, sb+nsl) of chunk c (width nsl*1024)."""
            w = nsl * SLAB
            jre, jim = c, c + CCH
            sl = slice(sb * SLAB, sb * SLAB + w)
            cs, sn = cosb[c][:, sl], sinb[c][:, sl]
            vre, vim = v[jre][:, sl], v[jim][:, sl]   # vim holds -Bu_im
            pool = scr1
            sA = pool.tile([P, w], F16, tag="s", name="sA")[:]
            sB = pool.tile([P, w], F16, tag="s", name="sB")[:]
            sC = pool.tile([P, w], F16, tag="s", name="sC")[:]
            nc.vector.tensor_mul(sA, cs, vre)        # cs*vre
            nc.vector.tensor_mul(sB, sn, vim)        # sn*vimN
            nc.vector.tensor_mul(sC, cs, vim)        # cs*vimN
            nc.vector.tensor_mul(vim, sn, vre)       # sn*vre -> vim slab
            nc.vector.tensor_sub(vre, sA, sB)        # vre' = a1-a2
            # vim'N = a3 + a4   (accum: vim (holds a4) += sC)
            nc.gpsimd.dma_start(vim, sC, accum_op=ADD)

        def scan_pair(c, sb):
            jre, jim = c, c + CCH
            sl = slice(sb * SLAB, (sb + 1) * SLAB)
            vre, vim = v[jre][:, sl], v[jim][:, sl]
            if c < 2:
                init_r = (0.0 if sb == 0
                          else v[jre][:, sb * SLAB - 1:sb * SLAB])
                init_i = (0.0 if sb == 0
                          else v[jim][:, sb * SLAB - 1:sb * SLAB])
                nc.vector.tensor_tensor_scan(vre, r_ap(c, SLAB), vre, init_r,
                                             op0=MULT, op1=ADD)
                nc.vector.tensor_tensor_scan(vim, r_ap(c, SLAB), vim, init_i,
                                             op0=MULT, op1=ADD)
            else:
                e = (sb + 1) * SLAB
                init_r = 0.0 if sb == NSLAB - 1 else v[jre][:, e:e + 1]
                init_i = 0.0 if sb == NSLAB - 1 else v[jim][:, e:e + 1]
                nc.vector.tensor_tensor_scan(vre[:, ::-1], r_ap(c, SLAB),
                                             vre[:, ::-1], init_r,
                                             op0=MULT, op1=ADD)
                nc.vector.tensor_tensor_scan(vim[:, ::-1], r_ap(c, SLAB),
                                             vim[:, ::-1], init_i,
                                             op0=MULT, op1=ADD)

        def untwiddle(c, sb, nsl=1):
            w = nsl * SLAB
            jre, jim = c, c + CCH
            sl = slice(sb * SLAB, sb * SLAB + w)
            cs, sn = cosb[c][:, sl], sinb[c][:, sl]
            wr, wi = v[jre][:, sl], v[jim][:, sl]   # in-place scan outputs
            pool = scr1
            s1 = pool.tile([P, w], F16, tag="s", name="s1")[:]
            s2 = pool.tile([P, w], F16, tag="s", name="s2")[:]
            s3 = pool.tile([P, w], F16, tag="s", name="s3")[:]
            nc.vector.tensor_mul(s1, cs, wr)         # cs*wr
            nc.vector.tensor_mul(s2, cs, wi)         # cs*wiN
            nc.vector.tensor_mul(cs, sn, wi)         # sn*wiN -> cs slab
            nc.vector.tensor_mul(s3, sn, wr)         # sn*wr
            # ximN = b4 - b3 -> sn slab  (v slabs keep w for chain inits)
            nc.vector.tensor_sub(sn, s2, s3)
            # xre = b2 + b1 -> cs slab  (accum: cs (holds b2) += s1)
            nc.gpsimd.dma_start(cs, s1, accum_op=ADD)

        # x source for yT matmul: plane k, slab sb, window tw (512 cols)
        def x_src(k, sb, tw):
            col = sb * SLAB + tw * SEG
            src = cosb[k] if k < CCH else sinb[k - CCH]
            return src[:, col:col + SEG]

        FWD_K = (0, 1, 4, 5)   # x chunks from fwd untwiddles
        BWD_K = (2, 3, 6, 7)
        _ytiles = {}

        def yT_slab_half(sb, ks, first):
            # two H-chunks share one 4-bank PSUM tile (halves of the free dim)
            for jp in range(H // P // 2):
                if first:
                    ps = bup.tile([P, 2 * SLAB], F32, tag="bup",
                                  name=f"y{sb}_{jp}")
                    _ytiles[(sb, jp)] = ps
                else:
                    ps = _ytiles[(sb, jp)]
                for jo in range(2):
                    j = 2 * jp + jo
                    for ki, k in enumerate(ks):
                        for tw in range(SLAB // SEG):
                            nc.tensor.matmul(
                                ps[:, jo * SLAB + tw * SEG:
                                   jo * SLAB + (tw + 1) * SEG],
                                cmat[k][:, j * P:(j + 1) * P],
                                x_src(k, sb, tw),
                                start=(first and ki == 0),
                                stop=((not first) and ki == len(ks) - 1),
                            )
                if not first:
                    yt = ysb.tile([P, 2 * SLAB], F16, tag="y", name="yt")
                    nc.scalar.copy(yt[:], ps[:])
                    for jo in range(2):
                        j = 2 * jp + jo
                        nc.sync.dma_start(
                            yT[j * P:(j + 1) * P, sb * SLAB:(sb + 1) * SLAB],
                            yt[:, jo * SLAB:(jo + 1) * SLAB])

        def yT_slab(sb):
            yT_slab_half(sb, FWD_K, True)
            yT_slab_half(sb, BWD_K, False)

        # ---- schedule: step s = fwd slab s (c=0,1), bwd slab 3-s;
        # wave emission, software-pipelined one step deep.
        def slab_of(c, s):
            return s if c < 2 else NSLAB - 1 - s

        def tw_wave(s):
            for c in range(CCH):
                twiddle_in(c, slab_of(c, s))

        def scan_wave(s):
            for c in range(CCH):
                scan_pair(c, slab_of(c, s))

        def untw_wave(s):
            for c in range(CCH):
                untwiddle(c, slab_of(c, s))

        tw_wave(0)
        scan_wave(0)
        tw_wave(1)
        untw_wave(0)
        scan_wave(1)
        tw_wave(2)
        untw_wave(1)
        scan_wave(2)
        tw_wave(3)
        untw_wave(2)
        yT_slab(1)
        yT_slab(2)
        scan_wave(3)
        yT_slab_half(3, BWD_K, True)
        untw_wave(3)
        yT_slab_half(3, FWD_K, False)
        yT_slab(0)

    nc.compile()
    return nc


def prepare_inputs(u, lengths, nu_log, theta_log, B, C):
    """Host-side prep. Heavy math in fp64 for accuracy."""
    u = np.asarray(u)
    lengths = np.asarray(lengths)
    nu = np.exp(np.asarray(nu_log, np.float64))
    theta = np.exp(np.asarray(theta_log, np.float64))
    r = np.exp(-nu)                                    # |lam|, (N,)

    j = np.arange(L, dtype=np.float64)
    half = N // 2
    # fwd channels: phase theta*t; bwd channels: phase theta*(L-1-t)
    phase = np.empty((N, L), np.float64)
    phase[:half] = theta[:half, None] * j[None, :]
    phase[half:] = theta[half:, None] * (L - 1 - j)[None, :]
    phase = np.mod(phase, 2 * np.pi)
    cos_base = np.cos(phase).astype(np.float16)
    sin_base = np.sin(phase).astype(np.float16)

    Bcat = np.empty((H, 2 * N), np.float16)
    Bcat[:, :N] = np.asarray(B)[..., 0]
    Bcat[:, N:] = -np.asarray(B)[..., 1]               # negated B_im
    Ccat = np.empty((2 * N, H), np.float16)
    Ccat[:N] = np.asarray(C)[0]
    Ccat[N:] = np.asarray(C)[1]                        # +C1 (x_imN carries -)
    rdec = r.reshape(CCH, P).T.astype(np.float32).copy()   # (128, 4)

    in_maps = []
    for b in range(BSZ):
        ln = int(lengths[b])
        ub = np.array(u[b], np.float32)
        if ln < L:
            ub[ln:, :] = 0.0
        uTh = np.ascontiguousarray(ub.T.astype(np.float16))
        cosb = cos_base.copy()
        sinb = sin_base.copy()
        if ln < L:
            cosb[:, ln:] = 0
            sinb[:, ln:] = 0
        in_maps.append({
            "uT": uTh, "cosT": cosb, "sinT": sinb,
            "rdec": rdec, "Bcat": Bcat, "Ccat": Ccat,
        })
    return in_maps


def kernel(u, lengths, nu_log, theta_log, B, C):
    global _CACHED
    from concourse.bass_utils import run_bass_kernel_spmd
    in_maps = prepare_inputs(u, lengths, nu_log, theta_log, B, C)
    if _CACHED is None:
        _CACHED = build_nc()
    res = run_bass_kernel_spmd(_CACHED, in_maps, list(range(BSZ)))
    y = np.stack(
        [res.results[i]["yT"].T.astype(np.float32) for i in range(BSZ)],
        axis=0)
    return y


# revision 9
# speedup vs baseline: 1.0430x; 1.0269x over previous
"""Bidirectional complex-diagonal LRU on 8 TRN2 cores — v2.

Key structure (vs the v1 kernel):
  - No data reversal anywhere: backward chunks scan via reversed APs
    (time-descending recurrence); tables for bwd chunks are host-baked with
    phase theta*(L-1-t) in forward storage order. All DVE tensor ops hit the
    2x perf mode (stride +1, fp16, SBUF).
  - Sign juggling (host-negated B_im, +C1, negated-w-im scan) turns one
    combine per complex multiply into a pure ADD -> rides SWDGE accum DMA.
    Per (chunk, slab): 8 muls + 2 subs on DVE, 2 accum-DMAs.
  - Scans run in place (w overwrites its v slab); cross-slab chain init is
    an AP into the neighbouring column.
  - Middle slabs (away from the chain heads) process twiddle/untwiddle at
    2048 width: ~12% cheaper per element and half the instructions/accums.
  - y computed transposed (yT = C^T x, [H, L], fp16); host transposes back.
    x chunks are the matmul MOVING operand -> stationary C chunks reused.
  - y(3) opens early with its bwd half (ready after step 0); fwd closes it.

Self-contained: hardcodes B=8, L=4096, H=N=512, 8 cores.
"""

import numpy as np
from contextlib import ExitStack

import concourse.bass as bass
import concourse.bacc as bacc
import concourse.mybir as mybir
import concourse.tile as tile

P = 128
L = 4096
H = 512
N = 512
BSZ = 8
SEG = 512
SLAB = 1024
NSLAB = L // SLAB        # 4
KH = H // P              # 4
NCH = 2 * N // P         # 8 real-channel planes (re 0..3, imN 4..7)
CCH = N // P             # 4 complex chunks (0,1 fwd; 2,3 bwd)

F16 = mybir.dt.float16
F32 = mybir.dt.float32
MULT = mybir.AluOpType.mult
ADD = mybir.AluOpType.add

_CACHED = None


def build_nc():
    nc = bacc.Bacc("TRN2", target_bir_lowering=False, debug=False)
    uT = nc.declare_dram_parameter("uT", [H, L], F16, isOutput=False)
    cosT = nc.declare_dram_parameter("cosT", [N, L], F16, isOutput=False)
    sinT = nc.declare_dram_parameter("sinT", [N, L], F16, isOutput=False)
    rdec = nc.declare_dram_parameter("rdec", [P, CCH], F32, isOutput=False)
    Bcat = nc.declare_dram_parameter("Bcat", [H, 2 * N], F16, isOutput=False)
    Ccat = nc.declare_dram_parameter("Ccat", [2 * N, H], F16, isOutput=False)
    yT = nc.declare_dram_parameter("yT", [H, L], F16, isOutput=True)

    with tile.TileContext(nc) as tc, ExitStack() as ctx:
        const = ctx.enter_context(tc.tile_pool(name="const", bufs=1))
        big = ctx.enter_context(tc.tile_pool(name="big", bufs=1))
        upool = ctx.enter_context(tc.tile_pool(name="upool", bufs=1))
        scr1 = ctx.enter_context(tc.tile_pool(name="scr1", bufs=12))
        ysb = ctx.enter_context(tc.tile_pool(name="ysb", bufs=2))
        bup = ctx.enter_context(tc.tile_pool(name="bup", bufs=2, space="PSUM"))

        u_t = [upool.tile([P, 2 * SLAB], F16, tag=f"uT{k}", name=f"uT{k}")
               for k in range(KH)]
        cosb = [big.tile([P, L], F16, tag=f"cos{c}", name=f"cos{c}")
                for c in range(CCH)]
        sinb = [big.tile([P, L], F16, tag=f"sin{c}", name=f"sin{c}")
                for c in range(CCH)]
        v = [big.tile([P, L], F16, tag=f"v{j}", name=f"v{j}") for j in range(NCH)]
        bmat = [const.tile([P, 2 * N], F16, tag=f"B{k}", name=f"Bm{k}")
                for k in range(KH)]
        cmat = [const.tile([P, H], F16, tag=f"C{k}", name=f"Cm{k}")
                for k in range(NCH)]
        rdec_t = const.tile([P, CCH], F32, tag="rdec", name="rdec_t")

        # ---- constant DMAs: Bcat + u wave 1 first so Bu starts ASAP
        nc.sync.dma_start(rdec_t[:], rdec[:])
        for k in range(KH):
            nc.sync.dma_start(bmat[k][:], Bcat[k * P:(k + 1) * P, :])
        # u piece A: t cols [0:1024) + [3072:4096) — on the Activation
        # HWDGE queue so they run in parallel with bmat/tables on sync
        for k in range(KH):
            nc.scalar.dma_start(u_t[k][:, 0:SLAB],
                                uT[k * P:(k + 1) * P, 0:SLAB])
        for k in range(KH):
            nc.scalar.dma_start(u_t[k][:, SLAB:2 * SLAB],
                                uT[k * P:(k + 1) * P, 3 * SLAB:4 * SLAB])
        # tables: slab 0 (fwd) + slab 3 (bwd scan heads) first
        for c in range(CCH):
            sl0 = slice(0, SLAB) if c < 2 else slice(3 * SLAB, 4 * SLAB)
            nc.sync.dma_start(cosb[c][:, sl0], cosT[c * P:(c + 1) * P, sl0])
            nc.sync.dma_start(sinb[c][:, sl0], sinT[c * P:(c + 1) * P, sl0])
        for c in range(CCH):
            slr = slice(SLAB, 4 * SLAB) if c < 2 else slice(0, 3 * SLAB)
            nc.sync.dma_start(cosb[c][:, slr], cosT[c * P:(c + 1) * P, slr])
            nc.sync.dma_start(sinb[c][:, slr], sinT[c * P:(c + 1) * P, slr])
        for k in range(NCH):
            nc.sync.dma_start(cmat[k][:], Ccat[k * P:(k + 1) * P, :])

        # ---- Phase A: Bu matmuls ----
        # group = 4 tsegs of one plane in a 4-bank PSUM tile; k-inner reuses
        # the stationary across the 4 tsegs.
        # utile cols: piece A: slab0 -> [0:1024), slab3 -> [1024:2048)
        #             piece B: slab1 -> [0:1024), slab2 -> [1024:2048)
        NCH_ORDER = [0, 4, 1, 5, 2, 6, 3, 7]   # pair c ready in order 0,1,2,3

        def bu_group(nch, piece):
            # piece 0 covers slabs (0, 3); piece 1 covers (1, 2)
            slabs = (0, 3) if piece == 0 else (1, 2)
            ps = bup.tile([P, 4 * SEG], F32, tag="bup", name=f"bu{nch}_{piece}")
            for k in range(KH):
                for i in range(4):
                    nc.tensor.matmul(
                        ps[:, i * SEG:(i + 1) * SEG],
                        bmat[k][:, nch * P:(nch + 1) * P],
                        u_t[k][:, i * SEG:(i + 1) * SEG],
                        start=(k == 0), stop=(k == KH - 1),
                    )
            nc.scalar.copy(v[nch][:, slabs[0] * SLAB:(slabs[0] + 1) * SLAB],
                           ps[:, 0:SLAB])
            nc.scalar.copy(v[nch][:, slabs[1] * SLAB:(slabs[1] + 1) * SLAB],
                           ps[:, SLAB:2 * SLAB])

        for nch in NCH_ORDER:
            bu_group(nch, 0)
        # u piece B: slab1 cols -> utile[0:1024), slab2 cols -> [1024:2048)
        for k in range(KH):
            nc.scalar.dma_start(u_t[k][:, 0:SLAB],
                                uT[k * P:(k + 1) * P, SLAB:2 * SLAB])
        for k in range(KH):
            nc.scalar.dma_start(u_t[k][:, SLAB:2 * SLAB],
                                uT[k * P:(k + 1) * P, 2 * SLAB:3 * SLAB])
        for nch in NCH_ORDER:
            bu_group(nch, 1)

        # ---- Phases B/C/D ----
        def r_ap(c, w):
            return rdec_t[:, c:c + 1].broadcast_to((P, w))

        def twiddle_in(c, sb, nsl=1):
            """Twiddle slabs [sb, sb+nsl) of chunk c (width nsl*1024)."""
            w = nsl * SLAB
            jre, jim = c, c + CCH
            sl = slice(sb * SLAB, sb * SLAB + w)
            cs, sn = cosb[c][:, sl], sinb[c][:, sl]
            vre, vim = v[jre][:, sl], v[jim][:, sl]   # vim holds -Bu_im
            pool = scr1
            sA = pool.tile([P, w], F16, tag="s", name="sA")[:]
            sB = pool.tile([P, w], F16, tag="s", name="sB")[:]
            sC = pool.tile([P, w], F16, tag="s", name="sC")[:]
            nc.vector.tensor_mul(sA, cs, vre)        # cs*vre
            nc.vector.tensor_mul(sB, sn, vim)        # sn*vimN
            nc.vector.tensor_mul(sC, cs, vim)        # cs*vimN
            nc.vector.tensor_mul(vim, sn, vre)       # sn*vre -> vim slab
            nc.vector.tensor_sub(vre, sA, sB)        # vre' = a1-a2
            # vim'N = a3 + a4   (accum: vim (holds a4) += sC)
            nc.gpsimd.dma_start(vim, sC, accum_op=ADD)

        def scan_pair(c, sb):
            jre, jim = c, c + CCH
            sl = slice(sb * SLAB, (sb + 1) * SLAB)
            vre, vim = v[jre][:, sl], v[jim][:, sl]
            if c < 2:
                init_r = (0.0 if sb == 0
                          else v[jre][:, sb * SLAB - 1:sb * SLAB])
                init_i = (0.0 if sb == 0
                          else v[jim][:, sb * SLAB - 1:sb * SLAB])
                nc.vector.tensor_tensor_scan(vre, r_ap(c, SLAB), vre, init_r,
                                             op0=MULT, op1=ADD)
                nc.vector.tensor_tensor_scan(vim, r_ap(c, SLAB), vim, init_i,
                                             op0=MULT, op1=ADD)
            else:
                e = (sb + 1) * SLAB
                init_r = 0.0 if sb == NSLAB - 1 else v[jre][:, e:e + 1]
                init_i = 0.0 if sb == NSLAB - 1 else v[jim][:, e:e + 1]
                nc.vector.tensor_tensor_scan(vre[:, ::-1], r_ap(c, SLAB),
                                             vre[:, ::-1], init_r,
                                             op0=MULT, op1=ADD)
                nc.vector.tensor_tensor_scan(vim[:, ::-1], r_ap(c, SLAB),
                                             vim[:, ::-1], init_i,
                                             op0=MULT, op1=ADD)

        def untwiddle(c, sb, nsl=1):
            w = nsl * SLAB
            jre, jim = c, c + CCH
            sl = slice(sb * SLAB, sb * SLAB + w)
            cs, sn = cosb[c][:, sl], sinb[c][:, sl]
            wr, wi = v[jre][:, sl], v[jim][:, sl]   # in-place scan outputs
            pool = scr1
            s1 = pool.tile([P, w], F16, tag="s", name="s1")[:]
            s2 = pool.tile([P, w], F16, tag="s", name="s2")[:]
            s3 = pool.tile([P, w], F16, tag="s", name="s3")[:]
            nc.vector.tensor_mul(s1, cs, wr)         # cs*wr
            nc.vector.tensor_mul(s2, cs, wi)         # cs*wiN
            nc.vector.tensor_mul(cs, sn, wi)         # sn*wiN -> cs slab
            nc.vector.tensor_mul(s3, sn, wr)         # sn*wr
            # ximN = b4 - b3 -> sn slab  (v slabs keep w for chain inits)
            nc.vector.tensor_sub(sn, s2, s3)
            # xre = b2 + b1 -> cs slab  (accum: cs (holds b2) += s1)
            nc.gpsimd.dma_start(cs, s1, accum_op=ADD)

        # x source for yT matmul: plane k, slab sb, window tw (512 cols)
        def x_src(k, sb, tw):
            col = sb * SLAB + tw * SEG
            src = cosb[k] if k < CCH else sinb[k - CCH]
            return src[:, col:col + SEG]

        FWD_K = (0, 1, 4, 5)   # x chunks from fwd untwiddles
        BWD_K = (2, 3, 6, 7)
        _ytiles = {}

        def yT_slab_half(sb, ks, first):
            # two H-chunks share one 4-bank PSUM tile (halves of the free dim)
            for jp in range(H // P // 2):
                if first:
                    ps = bup.tile([P, 2 * SLAB], F32, tag="bup",
                                  name=f"y{sb}_{jp}")
                    _ytiles[(sb, jp)] = ps
                else:
                    ps = _ytiles[(sb, jp)]
                for jo in range(2):
                    j = 2 * jp + jo
                    for ki, k in enumerate(ks):
                        for tw in range(SLAB // SEG):
                            nc.tensor.matmul(
                                ps[:, jo * SLAB + tw * SEG:
                                   jo * SLAB + (tw + 1) * SEG],
                                cmat[k][:, j * P:(j + 1) * P],
                                x_src(k, sb, tw),
                                start=(first and ki == 0),
                                stop=((not first) and ki == len(ks) - 1),
                            )
                if not first:
                    yt = ysb.tile([P, 2 * SLAB], F16, tag="y", name="yt")
                    nc.scalar.copy(yt[:], ps[:])
                    for jo in range(2):
                        j = 2 * jp + jo
                        nc.sync.dma_start(
                            yT[j * P:(j + 1) * P, sb * SLAB:(sb + 1) * SLAB],
                            yt[:, jo * SLAB:(jo + 1) * SLAB])

        def yT_slab(sb):
            yT_slab_half(sb, FWD_K, True)
            yT_slab_half(sb, BWD_K, False)

        # ---- schedule: step s = fwd slab s (c=0,1), bwd slab 3-s;
        # wave emission, software-pipelined one step deep.
        def slab_of(c, s):
            return s if c < 2 else NSLAB - 1 - s

        def tw_wave(s):
            for c in range(CCH):
                twiddle_in(c, slab_of(c, s))

        def scan_wave(s):
            for c in range(CCH):
                scan_pair(c, slab_of(c, s))

        def untw_wave(s):
            for c in range(CCH):
                untwiddle(c, slab_of(c, s))

        tw_wave(0)
        scan_wave(0)
        tw_wave(1)
        untw_wave(0)
        scan_wave(1)
        tw_wave(2)
        untw_wave(1)
        scan_wave(2)
        tw_wave(3)
        untw_wave(2)
        yT_slab(1)
        yT_slab(2)
        scan_wave(3)
        yT_slab_half(3, BWD_K, True)
        untw_wave(3)
        yT_slab_half(3, FWD_K, False)
        yT_slab(0)

    nc.compile()
    return nc


def prepare_inputs(u, lengths, nu_log, theta_log, B, C):
    """Host-side prep. Heavy math in fp64 for accuracy."""
    u = np.asarray(u)
    lengths = np.asarray(lengths)
    nu = np.exp(np.asarray(nu_log, np.float64))
    theta = np.exp(np.asarray(theta_log, np.float64))
    r = np.exp(-nu)                                    # |lam|, (N,)

    j = np.arange(L, dtype=np.float64)
    half = N // 2
    # fwd channels: phase theta*t; bwd channels: phase theta*(L-1-t)
    phase = np.empty((N, L), np.float64)
    phase[:half] = theta[:half, None] * j[None, :]
    phase[half:] = theta[half:, None] * (L - 1 - j)[None, :]
    phase = np.mod(phase, 2 * np.pi)
    cos_base = np.cos(phase).astype(np.float16)
    sin_base = np.sin(phase).astype(np.float16)

    Bcat = np.empty((H, 2 * N), np.float16)
    Bcat[:, :N] = np.asarray(B)[..., 0]
    Bcat[:, N:] = -np.asarray(B)[..., 1]               # negated B_im
    Ccat = np.empty((2 * N, H), np.float16)
    Ccat[:N] = np.asarray(C)[0]
    Ccat[N:] = np.asarray(C)[1]                        # +C1 (x_imN carries -)
    rdec = r.reshape(CCH, P).T.astype(np.float32).copy()   # (128, 4)

    in_maps = []
    for b in range(BSZ):
        ln = int(lengths[b])
        ub = np.array(u[b], np.float32)
        if ln < L:
            ub[ln:, :] = 0.0
        uTh = np.ascontiguousarray(ub.T.astype(np.float16))
        cosb = cos_base.copy()
        sinb = sin_base.copy()
        if ln < L:
            cosb[:, ln:] = 0
            sinb[:, ln:] = 0
        in_maps.append({
            "uT": uTh, "cosT": cosb, "sinT": sinb,
            "rdec": rdec, "Bcat": Bcat, "Ccat": Ccat,
        })
    return in_maps


def kernel(u, lengths, nu_log, theta_log, B, C):
    global _CACHED
    from concourse.bass_utils import run_bass_kernel_spmd
    in_maps = prepare_inputs(u, lengths, nu_log, theta_log, B, C)
    if _CACHED is None:
        _CACHED = build_nc()
    res = run_bass_kernel_spmd(_CACHED, in_maps, list(range(BSZ)))
    y = np.stack(
        [res.results[i]["yT"].T.astype(np.float32) for i in range(BSZ)],
        axis=0)
    return y
